# revision 18
# baseline (speedup 1.0000x reference)
"""Deformable RoI pooling (deform_psroi_pooling, group_size=1) on 8 Trainium2
NeuronCores via Bass/Tile.

Strategy (v2)
-------------
Per roi r and output bin (ph, pw) the reference computes a weighted sum of
feature-map cells; folding bilinear weights, validity masking and 1/cnt into
a per-roi sparse matrix S over touched cells, each roi's output is

    out[r, :, bin] = sum_{cells q} S_r[q, bin] * F[b_r, :, q]

Device layout:
  * both images shipped channel-last as quad-cell rows [2*15200+2, 1024]
    bf16 (4 consecutive cells x 256 channels = 2KB per row, 2 zero pad
    rows at the end),
  * each roi's sorted quad list is decomposed into runs of consecutive
    quads, then segments of <= 2 consecutive quads; one segment = one
    partition slot; the indirect gather fetches K=2 consecutive rows per
    offset (4KB per slot),
  * per core, its rois' segment lists are packed back-to-back (at most 2
    rois per 128-slot chunk, padded only when a 3rd roi would enter a
    chunk; padding offsets are OOB so the gather skips them),
  * per chunk: one indirect-DMA gather of 128 x 2 quad rows, then 8 bf16
    matmuls (lhsT = S slice [128, 98] covering the chunk's <=2 rois in two
    49-bin parity blocks) into a [98, 256] fp32 PSUM tile,
  * PSUM -> SBUF (bf16) -> HBM, one DMA per group of G chunks,
  * host sums per-roi partials across chunks in fp32.

RoIs are globally balanced across all 8 cores by segment count (LPT
greedy); every core runs the identical program parameterised only by the
chunk count C.
"""

import numpy as np

P = 7          # pooled size (== part size)
SPP = 4        # samples per part
SPATIAL_SCALE = np.float32(0.0625)
TRANS_STD = np.float32(0.1)
N_IMG, C_FEAT, H_FEAT, W_FEAT = 2, 256, 200, 304
QUAD = 4                                  # cells per quad row
KSEG = 2                                  # consecutive quad rows per segment
NQROWS = H_FEAT * W_FEAT // QUAD          # 15200 quad rows per image
NROWS_ALL = N_IMG * NQROWS                # both images stacked
ROW_ELEMS = QUAD * C_FEAT                 # 1024 elems per quad row
SEG_ELEMS = KSEG * ROW_ELEMS              # 2048 elems per gathered slot
SEG_CELLS = KSEG * QUAD                   # 8 cells per slot
NBINS = P * P                             # 49
MBLK = 2 * NBINS                          # 98: two parity blocks of bins
N_CORES = 8
CHUNK = 128                               # segment slots per gather chunk
GOUT = 4                                  # chunks per output DMA
GCH = 3                                   # chunks per dma_gather instruction

_f32 = np.float32


def _host_tables(rois: np.ndarray, offset: np.ndarray):
    """Mirror the reference position math bit-exactly in float32 and build,
    per roi: the sorted list of global quad-row ids it touches and the dense
    weight matrix S [nquads, QUAD, NBINS] (weights already / max(cnt,1))."""
    R = rois.shape[0]
    rois = rois.astype(np.float32, copy=False)
    offset = offset.astype(np.float32, copy=False)

    b = rois[:, 0].astype(np.int32)
    roi_start_w = np.round(rois[:, 1]) * SPATIAL_SCALE - _f32(0.5)
    roi_start_h = np.round(rois[:, 2]) * SPATIAL_SCALE - _f32(0.5)
    roi_end_w = (np.round(rois[:, 3]) + _f32(1.0)) * SPATIAL_SCALE - _f32(0.5)
    roi_end_h = (np.round(rois[:, 4]) + _f32(1.0)) * SPATIAL_SCALE - _f32(0.5)
    roi_w = np.maximum(roi_end_w - roi_start_w, _f32(0.1))
    roi_h = np.maximum(roi_end_h - roi_start_h, _f32(0.1))
    bin_w = roi_w / _f32(P)
    bin_h = roi_h / _f32(P)
    sub_w = bin_w / _f32(SPP)
    sub_h = bin_h / _f32(SPP)

    ph = np.arange(P, dtype=np.float32)
    pw = np.arange(P, dtype=np.float32)
    # part_h == ph, part_w == pw for PART == P
    tx = offset[:, 0] * TRANS_STD                       # [R, P, P]
    ty = offset[:, 1] * TRANS_STD

    wstart = (pw[None, None, :] * bin_w[:, None, None]
              + roi_start_w[:, None, None] + tx * roi_w[:, None, None])
    hstart = (ph[None, :, None] * bin_h[:, None, None]
              + roi_start_h[:, None, None] + ty * roi_h[:, None, None])

    s = np.arange(SPP, dtype=np.float32)
    wpos = wstart[..., None, None] + s[None, None, None, None, :] * sub_w[:, None, None, None, None]
    hpos = hstart[..., None, None] + s[None, None, None, :, None] * sub_h[:, None, None, None, None]

    W = W_FEAT
    H = H_FEAT
    valid = ((wpos > _f32(-0.5)) & (wpos < _f32(W) - _f32(0.5))
             & (hpos > _f32(-0.5)) & (hpos < _f32(H) - _f32(0.5)))
    wc = np.clip(wpos, _f32(0.0), _f32(W - 1.0))
    hc = np.clip(hpos, _f32(0.0), _f32(H - 1.0))
    x0 = np.floor(wc)
    y0 = np.floor(hc)
    dx = wc - x0
    dy = hc - y0
    x0i = x0.astype(np.int32)
    y0i = y0.astype(np.int32)
    x1i = np.minimum(x0i + 1, W - 1)
    y1i = np.minimum(y0i + 1, H - 1)

    cnt = valid.sum(axis=(-1, -2)).astype(np.float32)           # [R, P, P]
    inv = _f32(1.0) / np.maximum(cnt, _f32(1.0))

    one = _f32(1.0)
    w00 = (one - dx) * (one - dy)
    w01 = dx * (one - dy)
    w10 = (one - dx) * dy
    w11 = dx * dy

    bins = np.broadcast_to(
        (np.arange(P)[:, None] * P + np.arange(P)[None, :])[None, :, :, None, None],
        valid.shape,
    )
    scale = np.broadcast_to(inv[:, :, :, None, None], valid.shape)

    per_roi = []
    for r in range(R):
        v = valid[r].ravel()
        if not v.any():
            per_roi.append((np.zeros(0, np.int32),
                            np.zeros((0, QUAD, NBINS), np.float32)))
            continue
        shp = valid[r].shape
        bc = lambda a: np.broadcast_to(a, shp).ravel()[v]
        sc = bc(scale[r]).astype(np.float32)
        bn = bc(bins[r]).astype(np.int64)
        cy0 = bc(y0i[r]).astype(np.int64)
        cy1 = bc(y1i[r]).astype(np.int64)
        cx0 = bc(x0i[r]).astype(np.int64)
        cx1 = bc(x1i[r]).astype(np.int64)
        ws = [bc(w00[r]) * sc, bc(w01[r]) * sc,
              bc(w10[r]) * sc, bc(w11[r]) * sc]
        cells = [cy0 * W + cx0, cy0 * W + cx1, cy1 * W + cx0, cy1 * W + cx1]

        cell_all = np.concatenate(cells)
        w_all = np.concatenate(ws).astype(np.float64)
        bin_all = np.concatenate([bn] * 4)

        quads = np.unique(cell_all >> 2).astype(np.int32)       # sorted
        qpos = np.searchsorted(quads, cell_all >> 2)
        key = (qpos * QUAD + (cell_all & 3)) * NBINS + bin_all
        S = np.bincount(key, weights=w_all,
                        minlength=len(quads) * QUAD * NBINS)
        S = S.astype(np.float32).reshape(len(quads), QUAD, NBINS)
        per_roi.append((quads + np.int32(b[r]) * np.int32(NQROWS), S))
    return per_roi


def _segments(per_roi):
    """Decompose each roi's sorted quad list into segments of <= KSEG
    consecutive quads. Returns per roi (seg_starts [m] int32,
    S8 [m, SEG_CELLS, NBINS] f32)."""
    out = []
    for quads, S in per_roi:
        if len(quads) == 0:
            out.append((np.zeros(0, np.int32),
                        np.zeros((0, SEG_CELLS, NBINS), np.float32)))
            continue
        run_bounds = np.where(np.diff(quads) != 1)[0] + 1
        starts_list = []
        s8_list = []
        for run_q, run_s in zip(np.split(quads, run_bounds),
                                np.split(S, run_bounds)):
            n = len(run_q)
            m = (n + KSEG - 1) // KSEG
            pad = m * KSEG - n
            starts_list.append(run_q[::KSEG])
            sp = np.concatenate(
                [run_s, np.zeros((pad, QUAD, NBINS), np.float32)], axis=0)
            s8_list.append(sp.reshape(m, SEG_CELLS, NBINS))
        out.append((np.concatenate(starts_list).astype(np.int32),
                    np.concatenate(s8_list, axis=0)))
    return out


def _balance(per_seg):
    """LPT greedy: assign rois to 8 cores balancing total segment count."""
    order = sorted(range(len(per_seg)),
                   key=lambda r: -len(per_seg[r][0]))
    loads = [0] * N_CORES
    core_rois = [[] for _ in range(N_CORES)]
    for rid in order:
        c = min(range(N_CORES), key=lambda i: loads[i])
        core_rois[c].append(rid)
        loads[c] += len(per_seg[rid][0])
    return core_rois


def _pack_core(rids, per_seg):
    """Pair rois (best-fit: minimize ceil((a+b)/CHUNK), then maximize fill)
    and place each pair chunk-aligned, so no chunk ever sees more than 2
    rois. Returns list of (rid, start_slot) and total slots used."""
    placements = [(rid, 0) for rid in rids if len(per_seg[rid][0]) == 0]
    items = [(len(per_seg[rid][0]), rid) for rid in rids
             if len(per_seg[rid][0]) > 0]
    items.sort(reverse=True)
    used = [False] * len(items)
    pairs = []
    for i, (a, rid_a) in enumerate(items):
        if used[i]:
            continue
        used[i] = True
        best_j, best_key = -1, None
        for j in range(len(items) - 1, i, -1):
            if used[j]:
                continue
            b = items[j][0]
            key = (-(-(a + b) // CHUNK), -(a + b))
            if best_key is None or key < best_key:
                best_key, best_j = key, j
        if best_j >= 0:
            used[best_j] = True
            pairs.append((rid_a, items[best_j][1]))
        else:
            pairs.append((rid_a, None))
    # local search: try re-pairing to reduce total chunk count / padding
    sz = {rid: len(per_seg[rid][0]) for _, rid in items}

    def cost(p):
        a, b = p
        s = sz[a] + (sz[b] if b is not None else 0)
        return -(-s // CHUNK) * CHUNK - s          # padding slots

    improved = True
    while improved:
        improved = False
        for i in range(len(pairs)):
            for j in range(i + 1, len(pairs)):
                a1, b1 = pairs[i]
                a2, b2 = pairs[j]
                if b1 is None or b2 is None:
                    continue
                cur = cost(pairs[i]) + cost(pairs[j])
                for p1, p2 in (((a1, a2), (b1, b2)), ((a1, b2), (a2, b1))):
                    if cost(p1) + cost(p2) < cur:
                        pairs[i], pairs[j] = p1, p2
                        improved = True
                        cur = cost(p1) + cost(p2)
    pos = 0
    for rid_a, rid_b in pairs:
        placements.append((rid_a, pos))
        sz = len(per_seg[rid_a][0])
        if rid_b is not None:
            placements.append((rid_b, pos + sz))
            sz += len(per_seg[rid_b][0])
        pos += -(-sz // CHUNK) * CHUNK
    return placements, pos


_PROGRAM_CACHE: dict = {}


def _build_program(C):
    """One SPMD Tile program for all 8 cores, parameterised by chunk count."""
    if C in _PROGRAM_CACHE:
        return _PROGRAM_CACHE[C]

    from concourse import bass, mybir, bacc
    from concourse.tile import TileContext
    from concourse.library_config import mlp

    nc = bacc.Bacc("TRN2", target_bir_lowering=False, debug=False,
                   num_devices=N_CORES)
    dataT = nc.declare_dram_parameter("dataT", [NROWS_ALL + KSEG, ROW_ELEMS],
                                      mybir.dt.bfloat16, isOutput=False)
    idxs = nc.declare_dram_parameter("idxs", [CHUNK, C * CHUNK // 16],
                                     mybir.dt.int16, isOutput=False)
    spack = nc.declare_dram_parameter("spack", [CHUNK, C * SEG_CELLS * MBLK],
                                      mybir.dt.bfloat16, isOutput=False)
    out = nc.declare_dram_parameter("out", [MBLK, C * C_FEAT],
                                    mybir.dt.bfloat16, isOutput=True)

    with TileContext(nc) as tc:
        with (
            tc.tile_pool(name="const", bufs=1) as cpool,
            tc.tile_pool(name="gt", bufs=4) as gpool,
            tc.tile_pool(name="ps", bufs=6, space="PSUM") as pspool,
            tc.tile_pool(name="ob", bufs=3) as opool,
        ):
            nc.gpsimd.load_library(mlp)
            idx_t = cpool.tile([CHUNK, C * CHUNK // 16], mybir.dt.int16)
            # idx load issued from gpsimd so it is not queued behind the
            # multi-MB spack loads on the sync engine's DMA ring
            nc.gpsimd.dma_start(out=idx_t[:], in_=idxs[:])
            s_t = cpool.tile([CHUNK, C * SEG_CELLS * MBLK], mybir.dt.bfloat16)
            # Load S per chunk-group so early matmuls can start sooner.
            scw = SEG_CELLS * MBLK
            for g in range(0, C, GOUT):
                hi = min(g + GOUT, C)
                nc.sync.dma_start(out=s_t[:, g * scw:hi * scw],
                                  in_=spack[:, g * scw:hi * scw])

            src_ap = bass.AP(dataT[:].tensor, 0,
                             [(ROW_ELEMS, NROWS_ALL + KSEG - 1),
                              (1, SEG_ELEMS)])

            ob = None
            gt = None
            for k in range(C):
                if k % GCH == 0:
                    ggw = min(GCH, C - k)
                    gt = gpool.tile([CHUNK, ggw * SEG_ELEMS], mybir.dt.bfloat16)
                    nidx = ggw * CHUNK
                    nc.gpsimd.dma_gather(
                        gt[:].rearrange("p (g e) -> p g e", e=SEG_ELEMS),
                        src_ap,
                        idx_t[:, k * CHUNK // 16:(k + ggw) * CHUNK // 16],
                        nidx,
                        nidx,
                        SEG_ELEMS,
                        elem_step=ROW_ELEMS,
                    )
                goff = (k % GCH) * SEG_ELEMS
                ps = pspool.tile([MBLK, C_FEAT], mybir.dt.float32)
                for e in range(SEG_CELLS):
                    nc.tensor.matmul(
                        ps[:],
                        lhsT=s_t[:, (k * SEG_CELLS + e) * MBLK:
                                 (k * SEG_CELLS + e + 1) * MBLK],
                        rhs=gt[:, goff + e * C_FEAT:goff + (e + 1) * C_FEAT],
                        start=(e == 0),
                        stop=(e == SEG_CELLS - 1),
                    )
                if k % GOUT == 0:
                    gw = min(GOUT, C - k)
                    ob = opool.tile([MBLK, gw * C_FEAT], mybir.dt.bfloat16)
                j = k % GOUT
                if k % 2 == 0:
                    nc.vector.tensor_copy(
                        out=ob[:, j * C_FEAT:(j + 1) * C_FEAT], in_=ps[:])
                else:
                    nc.scalar.copy(
                        out=ob[:, j * C_FEAT:(j + 1) * C_FEAT], in_=ps[:])
                if j == gw - 1:
                    k0 = k - j
                    nc.sync.dma_start(
                        out=out[:, k0 * C_FEAT:(k + 1) * C_FEAT], in_=ob[:])
    nc.compile()
    _PROGRAM_CACHE[C] = nc
    return nc


def _to_bf16(x):
    import ml_dtypes
    return x.astype(ml_dtypes.bfloat16)


def _core_inputs(per_seg, core_rois, C, dataT_b):
    in_maps = []
    chunk_maps = []          # per core: list over chunks of [rid_b0, rid_b1]
    import ml_dtypes
    for c in range(N_CORES):
        placements, _ = _pack_core(core_rois[c], per_seg)
        # linear index stream; pads fetch the zero row at NROWS_ALL
        idx_lin = np.full(C * CHUNK, NROWS_ALL, np.int16)
        spack = np.zeros((CHUNK, C * SEG_CELLS * MBLK), np.float32)
        cmap = [[-1, -1] for _ in range(C)]
        nplaced = 0
        for rid, start in placements:
            starts, S8 = per_seg[rid]
            q = len(starts)
            if q == 0:
                continue
            beta = nplaced % 2
            nplaced += 1
            rows = np.arange(start, start + q)
            idx_lin[rows] = starts
            ck = rows // CHUNK
            pp = rows % CHUNK
            # scatter S8 into spack: col = (ck*SEG_CELLS + e)*MBLK + beta*49
            for e in range(SEG_CELLS):
                cols = (ck * SEG_CELLS + e) * MBLK + beta * NBINS
                spack[pp[:, None], cols[:, None] + np.arange(NBINS)[None, :]] = S8[:, e, :]
            for kk in np.unique(ck):
                cmap[kk][beta] = rid
        # wrap: index j -> partition j%16, column j//16; replicate to the
        # eight 16-partition blocks
        idx_wrap = idx_lin.reshape(C * CHUNK // 16, 16).T   # [16, cols]
        idxs = np.tile(idx_wrap, (8, 1)).astype(np.int16)   # [128, cols]
        in_maps.append({
            "dataT": dataT_b,
            "idxs": idxs,
            "spack": spack.astype(ml_dtypes.bfloat16),
        })
        chunk_maps.append(cmap)
    return in_maps, chunk_maps


def kernel(data: np.ndarray, rois: np.ndarray, offset: np.ndarray) -> np.ndarray:
    from concourse.bass_utils import run_bass_kernel_spmd

    data = np.ascontiguousarray(data, dtype=np.float32)
    rois = np.asarray(rois, dtype=np.float32)
    offset = np.asarray(offset, dtype=np.float32)
    R = rois.shape[0]

    per_roi = _host_tables(rois, offset)
    per_seg = _segments(per_roi)
    core_rois = _balance(per_seg)
    C = 0
    for c in range(N_CORES):
        _, rows = _pack_core(core_rois[c], per_seg)
        C = max(C, (rows + CHUNK - 1) // CHUNK)
    nc = _build_program(C)

    # channel-last quad-row layout, both images stacked + KSEG zero pad rows
    dataT = np.concatenate([
        np.ascontiguousarray(data[i].transpose(1, 2, 0)).reshape(NQROWS, ROW_ELEMS)
        for i in range(N_IMG)
    ] + [np.zeros((KSEG, ROW_ELEMS), np.float32)], axis=0)
    dataT_b = _to_bf16(dataT)
    in_maps, chunk_maps = _core_inputs(per_seg, core_rois, C, dataT_b)

    res = run_bass_kernel_spmd(nc, in_maps, list(range(N_CORES)), trace=False)

    out_full = np.zeros((R, C_FEAT, P, P), np.float32)
    for c in range(N_CORES):
        o = np.asarray(res.results[c]["out"]).astype(np.float32)  # [98, C*256]
        o = o.reshape(MBLK, C, C_FEAT).transpose(1, 0, 2)         # [C, 98, 256]
        for k in range(C):
            for beta in range(2):
                rid = chunk_maps[c][k][beta]
                if rid >= 0:
                    blk = o[k, beta * NBINS:(beta + 1) * NBINS]   # [49, 256]
                    out_full[rid] += blk.T.reshape(C_FEAT, P, P)
    return out_full


# revision 20
# speedup vs baseline: 1.0509x; 1.0509x over previous
"""Deformable RoI pooling (deform_psroi_pooling, group_size=1) on 8 Trainium2
NeuronCores via Bass/Tile.

Strategy (v2)
-------------
Per roi r and output bin (ph, pw) the reference computes a weighted sum of
feature-map cells; folding bilinear weights, validity masking and 1/cnt into
a per-roi sparse matrix S over touched cells, each roi's output is

    out[r, :, bin] = sum_{cells q} S_r[q, bin] * F[b_r, :, q]

Device layout:
  * both images shipped channel-last as quad-cell rows [2*15200+2, 1024]
    bf16 (4 consecutive cells x 256 channels = 2KB per row, 2 zero pad
    rows at the end),
  * each roi's sorted quad list is decomposed into runs of consecutive
    quads, then segments of <= 2 consecutive quads; one segment = one
    partition slot; the indirect gather fetches K=2 consecutive rows per
    offset (4KB per slot),
  * per core, its rois' segment lists are packed back-to-back (at most 2
    rois per 128-slot chunk, padded only when a 3rd roi would enter a
    chunk; padding offsets are OOB so the gather skips them),
  * per chunk: one indirect-DMA gather of 128 x 2 quad rows, then 8 bf16
    matmuls (lhsT = S slice [128, 98] covering the chunk's <=2 rois in two
    49-bin parity blocks) into a [98, 256] fp32 PSUM tile,
  * PSUM -> SBUF (bf16) -> HBM, one DMA per group of G chunks,
  * host sums per-roi partials across chunks in fp32.

RoIs are globally balanced across all 8 cores by segment count (LPT
greedy); every core runs the identical program parameterised only by the
chunk count C.
"""

import numpy as np

P = 7          # pooled size (== part size)
SPP = 4        # samples per part
SPATIAL_SCALE = np.float32(0.0625)
TRANS_STD = np.float32(0.1)
N_IMG, C_FEAT, H_FEAT, W_FEAT = 2, 256, 200, 304
QUAD = 4                                  # cells per quad row
KSEG = 2                                  # consecutive quad rows per segment
NQROWS = H_FEAT * W_FEAT // QUAD          # 15200 quad rows per image
NROWS_ALL = N_IMG * NQROWS                # both images stacked
ROW_ELEMS = QUAD * C_FEAT                 # 1024 elems per quad row
SEG_ELEMS = KSEG * ROW_ELEMS              # 2048 elems per gathered slot
SEG_CELLS = KSEG * QUAD                   # 8 cells per slot
NBINS = P * P                             # 49
MBLK = 2 * NBINS                          # 98: two parity blocks of bins
N_CORES = 8
CHUNK = 128                               # segment slots per gather chunk
GOUT = 4                                  # chunks per output DMA
GCH = 3                                   # chunks per dma_gather instruction

_f32 = np.float32


def _host_tables(rois: np.ndarray, offset: np.ndarray):
    """Mirror the reference position math bit-exactly in float32 and build,
    per roi: the sorted list of global quad-row ids it touches and the dense
    weight matrix S [nquads, QUAD, NBINS] (weights already / max(cnt,1))."""
    R = rois.shape[0]
    rois = rois.astype(np.float32, copy=False)
    offset = offset.astype(np.float32, copy=False)

    b = rois[:, 0].astype(np.int32)
    roi_start_w = np.round(rois[:, 1]) * SPATIAL_SCALE - _f32(0.5)
    roi_start_h = np.round(rois[:, 2]) * SPATIAL_SCALE - _f32(0.5)
    roi_end_w = (np.round(rois[:, 3]) + _f32(1.0)) * SPATIAL_SCALE - _f32(0.5)
    roi_end_h = (np.round(rois[:, 4]) + _f32(1.0)) * SPATIAL_SCALE - _f32(0.5)
    roi_w = np.maximum(roi_end_w - roi_start_w, _f32(0.1))
    roi_h = np.maximum(roi_end_h - roi_start_h, _f32(0.1))
    bin_w = roi_w / _f32(P)
    bin_h = roi_h / _f32(P)
    sub_w = bin_w / _f32(SPP)
    sub_h = bin_h / _f32(SPP)

    ph = np.arange(P, dtype=np.float32)
    pw = np.arange(P, dtype=np.float32)
    # part_h == ph, part_w == pw for PART == P
    tx = offset[:, 0] * TRANS_STD                       # [R, P, P]
    ty = offset[:, 1] * TRANS_STD

    wstart = (pw[None, None, :] * bin_w[:, None, None]
              + roi_start_w[:, None, None] + tx * roi_w[:, None, None])
    hstart = (ph[None, :, None] * bin_h[:, None, None]
              + roi_start_h[:, None, None] + ty * roi_h[:, None, None])

    s = np.arange(SPP, dtype=np.float32)
    wpos = wstart[..., None, None] + s[None, None, None, None, :] * sub_w[:, None, None, None, None]
    hpos = hstart[..., None, None] + s[None, None, None, :, None] * sub_h[:, None, None, None, None]

    W = W_FEAT
    H = H_FEAT
    valid = ((wpos > _f32(-0.5)) & (wpos < _f32(W) - _f32(0.5))
             & (hpos > _f32(-0.5)) & (hpos < _f32(H) - _f32(0.5)))
    wc = np.clip(wpos, _f32(0.0), _f32(W - 1.0))
    hc = np.clip(hpos, _f32(0.0), _f32(H - 1.0))
    x0 = np.floor(wc)
    y0 = np.floor(hc)
    dx = wc - x0
    dy = hc - y0
    x0i = x0.astype(np.int32)
    y0i = y0.astype(np.int32)
    x1i = np.minimum(x0i + 1, W - 1)
    y1i = np.minimum(y0i + 1, H - 1)

    cnt = valid.sum(axis=(-1, -2)).astype(np.float32)           # [R, P, P]
    inv = _f32(1.0) / np.maximum(cnt, _f32(1.0))

    one = _f32(1.0)
    w00 = (one - dx) * (one - dy)
    w01 = dx * (one - dy)
    w10 = (one - dx) * dy
    w11 = dx * dy

    bins = np.broadcast_to(
        (np.arange(P)[:, None] * P + np.arange(P)[None, :])[None, :, :, None, None],
        valid.shape,
    )
    scale = np.broadcast_to(inv[:, :, :, None, None], valid.shape)

    per_roi = []
    for r in range(R):
        v = valid[r].ravel()
        if not v.any():
            per_roi.append((np.zeros(0, np.int32),
                            np.zeros((0, QUAD, NBINS), np.float32)))
            continue
        shp = valid[r].shape
        bc = lambda a: np.broadcast_to(a, shp).ravel()[v]
        sc = bc(scale[r]).astype(np.float32)
        bn = bc(bins[r]).astype(np.int64)
        cy0 = bc(y0i[r]).astype(np.int64)
        cy1 = bc(y1i[r]).astype(np.int64)
        cx0 = bc(x0i[r]).astype(np.int64)
        cx1 = bc(x1i[r]).astype(np.int64)
        ws = [bc(w00[r]) * sc, bc(w01[r]) * sc,
              bc(w10[r]) * sc, bc(w11[r]) * sc]
        cells = [cy0 * W + cx0, cy0 * W + cx1, cy1 * W + cx0, cy1 * W + cx1]

        cell_all = np.concatenate(cells)
        w_all = np.concatenate(ws).astype(np.float64)
        bin_all = np.concatenate([bn] * 4)

        quads = np.unique(cell_all >> 2).astype(np.int32)       # sorted
        qpos = np.searchsorted(quads, cell_all >> 2)
        key = (qpos * QUAD + (cell_all & 3)) * NBINS + bin_all
        S = np.bincount(key, weights=w_all,
                        minlength=len(quads) * QUAD * NBINS)
        S = S.astype(np.float32).reshape(len(quads), QUAD, NBINS)
        per_roi.append((quads + np.int32(b[r]) * np.int32(NQROWS), S))
    return per_roi


def _segments(per_roi):
    """Decompose each roi's sorted quad list into segments of <= KSEG
    consecutive quads. Returns per roi (seg_starts [m] int32,
    S8 [m, SEG_CELLS, NBINS] f32)."""
    out = []
    for quads, S in per_roi:
        if len(quads) == 0:
            out.append((np.zeros(0, np.int32),
                        np.zeros((0, SEG_CELLS, NBINS), np.float32)))
            continue
        run_bounds = np.where(np.diff(quads) != 1)[0] + 1
        starts_list = []
        s8_list = []
        for run_q, run_s in zip(np.split(quads, run_bounds),
                                np.split(S, run_bounds)):
            n = len(run_q)
            m = (n + KSEG - 1) // KSEG
            pad = m * KSEG - n
            starts_list.append(run_q[::KSEG])
            sp = np.concatenate(
                [run_s, np.zeros((pad, QUAD, NBINS), np.float32)], axis=0)
            s8_list.append(sp.reshape(m, SEG_CELLS, NBINS))
        out.append((np.concatenate(starts_list).astype(np.int32),
                    np.concatenate(s8_list, axis=0)))
    return out


def _balance(per_seg):
    """LPT greedy: assign rois to 8 cores balancing total segment count."""
    order = sorted(range(len(per_seg)),
                   key=lambda r: -len(per_seg[r][0]))
    loads = [0] * N_CORES
    core_rois = [[] for _ in range(N_CORES)]
    for rid in order:
        c = min(range(N_CORES), key=lambda i: loads[i])
        core_rois[c].append(rid)
        loads[c] += len(per_seg[rid][0])
    return core_rois


def _pack_core(rids, per_seg):
    """Pair rois (best-fit: minimize ceil((a+b)/CHUNK), then maximize fill)
    and place each pair chunk-aligned, so no chunk ever sees more than 2
    rois. Returns list of (rid, start_slot) and total slots used."""
    placements = [(rid, 0) for rid in rids if len(per_seg[rid][0]) == 0]
    items = [(len(per_seg[rid][0]), rid) for rid in rids
             if len(per_seg[rid][0]) > 0]
    items.sort(reverse=True)
    used = [False] * len(items)
    pairs = []
    for i, (a, rid_a) in enumerate(items):
        if used[i]:
            continue
        used[i] = True
        best_j, best_key = -1, None
        for j in range(len(items) - 1, i, -1):
            if used[j]:
                continue
            b = items[j][0]
            key = (-(-(a + b) // CHUNK), -(a + b))
            if best_key is None or key < best_key:
                best_key, best_j = key, j
        if best_j >= 0:
            used[best_j] = True
            pairs.append((rid_a, items[best_j][1]))
        else:
            pairs.append((rid_a, None))
    # local search: try re-pairing to reduce total chunk count / padding
    sz = {rid: len(per_seg[rid][0]) for _, rid in items}

    def cost(p):
        a, b = p
        s = sz[a] + (sz[b] if b is not None else 0)
        return -(-s // CHUNK) * CHUNK - s          # padding slots

    improved = True
    while improved:
        improved = False
        for i in range(len(pairs)):
            for j in range(i + 1, len(pairs)):
                a1, b1 = pairs[i]
                a2, b2 = pairs[j]
                if b1 is None or b2 is None:
                    continue
                cur = cost(pairs[i]) + cost(pairs[j])
                for p1, p2 in (((a1, a2), (b1, b2)), ((a1, b2), (a2, b1))):
                    if cost(p1) + cost(p2) < cur:
                        pairs[i], pairs[j] = p1, p2
                        improved = True
                        cur = cost(p1) + cost(p2)
    pos = 0
    for rid_a, rid_b in pairs:
        placements.append((rid_a, pos))
        sz = len(per_seg[rid_a][0])
        if rid_b is not None:
            placements.append((rid_b, pos + sz))
            sz += len(per_seg[rid_b][0])
        pos += -(-sz // CHUNK) * CHUNK
    return placements, pos


_PROGRAM_CACHE: dict = {}


def _build_program(C):
    """One SPMD Tile program for all 8 cores, parameterised by chunk count."""
    if C in _PROGRAM_CACHE:
        return _PROGRAM_CACHE[C]

    from concourse import bass, mybir, bacc
    from concourse.tile import TileContext
    from concourse.library_config import mlp

    nc = bacc.Bacc("TRN2", target_bir_lowering=False, debug=False,
                   num_devices=N_CORES)
    dataT = nc.declare_dram_parameter("dataT", [NROWS_ALL + KSEG, ROW_ELEMS],
                                      mybir.dt.bfloat16, isOutput=False)
    idxs = nc.declare_dram_parameter("idxs", [CHUNK, C * CHUNK // 16],
                                     mybir.dt.int16, isOutput=False)
    spack = nc.declare_dram_parameter("spack", [CHUNK, C * SEG_CELLS * MBLK],
                                      mybir.dt.bfloat16, isOutput=False)
    out = nc.declare_dram_parameter("out", [MBLK, C * C_FEAT],
                                    mybir.dt.bfloat16, isOutput=True)

    with TileContext(nc) as tc:
        with (
            tc.tile_pool(name="const", bufs=1) as cpool,
            tc.tile_pool(name="gt", bufs=4) as gpool,
            tc.tile_pool(name="ps", bufs=6, space="PSUM") as pspool,
            tc.tile_pool(name="ob", bufs=3) as opool,
        ):
            nc.gpsimd.load_library(mlp)
            idx_t = cpool.tile([CHUNK, C * CHUNK // 16], mybir.dt.int16)
            s_t = cpool.tile([CHUNK, C * SEG_CELLS * MBLK], mybir.dt.bfloat16)
            # load_library drains all in-flight DMA; gate the spack loads
            # (WAW on s_t) and the idx load behind it so the drain finds an
            # empty DMA subsystem and completes immediately
            nc.gpsimd.memzero(s_t[:, 0:2])
            nc.gpsimd.dma_start(out=idx_t[:], in_=idxs[:])
            # Load S per chunk-group so early matmuls can start sooner.
            scw = SEG_CELLS * MBLK
            for g in range(0, C, GOUT):
                hi = min(g + GOUT, C)
                nc.sync.dma_start(out=s_t[:, g * scw:hi * scw],
                                  in_=spack[:, g * scw:hi * scw])

            src_ap = bass.AP(dataT[:].tensor, 0,
                             [(ROW_ELEMS, NROWS_ALL + KSEG - 1),
                              (1, SEG_ELEMS)])

            ob = None
            gt = None
            for k in range(C):
                if k % GCH == 0:
                    ggw = min(GCH, C - k)
                    gt = gpool.tile([CHUNK, ggw * SEG_ELEMS], mybir.dt.bfloat16)
                    nidx = ggw * CHUNK
                    nc.gpsimd.dma_gather(
                        gt[:].rearrange("p (g e) -> p g e", e=SEG_ELEMS),
                        src_ap,
                        idx_t[:, k * CHUNK // 16:(k + ggw) * CHUNK // 16],
                        nidx,
                        nidx,
                        SEG_ELEMS,
                        elem_step=ROW_ELEMS,
                    )
                goff = (k % GCH) * SEG_ELEMS
                ps = pspool.tile([MBLK, C_FEAT], mybir.dt.float32)
                for e in range(SEG_CELLS):
                    nc.tensor.matmul(
                        ps[:],
                        lhsT=s_t[:, (k * SEG_CELLS + e) * MBLK:
                                 (k * SEG_CELLS + e + 1) * MBLK],
                        rhs=gt[:, goff + e * C_FEAT:goff + (e + 1) * C_FEAT],
                        start=(e == 0),
                        stop=(e == SEG_CELLS - 1),
                    )
                if k % GOUT == 0:
                    gw = min(GOUT, C - k)
                    ob = opool.tile([MBLK, gw * C_FEAT], mybir.dt.bfloat16)
                j = k % GOUT
                if k % 2 == 0:
                    nc.vector.tensor_copy(
                        out=ob[:, j * C_FEAT:(j + 1) * C_FEAT], in_=ps[:])
                else:
                    nc.scalar.copy(
                        out=ob[:, j * C_FEAT:(j + 1) * C_FEAT], in_=ps[:])
                if j == gw - 1:
                    k0 = k - j
                    nc.sync.dma_start(
                        out=out[:, k0 * C_FEAT:(k + 1) * C_FEAT], in_=ob[:])
    nc.compile()
    _PROGRAM_CACHE[C] = nc
    return nc


def _to_bf16(x):
    import ml_dtypes
    return x.astype(ml_dtypes.bfloat16)


def _core_inputs(per_seg, core_rois, C, dataT_b):
    in_maps = []
    chunk_maps = []          # per core: list over chunks of [rid_b0, rid_b1]
    import ml_dtypes
    for c in range(N_CORES):
        placements, _ = _pack_core(core_rois[c], per_seg)
        # linear index stream; pads fetch the zero row at NROWS_ALL
        idx_lin = np.full(C * CHUNK, NROWS_ALL, np.int16)
        spack = np.zeros((CHUNK, C * SEG_CELLS * MBLK), np.float32)
        cmap = [[-1, -1] for _ in range(C)]
        nplaced = 0
        for rid, start in placements:
            starts, S8 = per_seg[rid]
            q = len(starts)
            if q == 0:
                continue
            beta = nplaced % 2
            nplaced += 1
            rows = np.arange(start, start + q)
            idx_lin[rows] = starts
            ck = rows // CHUNK
            pp = rows % CHUNK
            # scatter S8 into spack: col = (ck*SEG_CELLS + e)*MBLK + beta*49
            for e in range(SEG_CELLS):
                cols = (ck * SEG_CELLS + e) * MBLK + beta * NBINS
                spack[pp[:, None], cols[:, None] + np.arange(NBINS)[None, :]] = S8[:, e, :]
            for kk in np.unique(ck):
                cmap[kk][beta] = rid
        # wrap: index j -> partition j%16, column j//16; replicate to the
        # eight 16-partition blocks
        idx_wrap = idx_lin.reshape(C * CHUNK // 16, 16).T   # [16, cols]
        idxs = np.tile(idx_wrap, (8, 1)).astype(np.int16)   # [128, cols]
        in_maps.append({
            "dataT": dataT_b,
            "idxs": idxs,
            "spack": spack.astype(ml_dtypes.bfloat16),
        })
        chunk_maps.append(cmap)
    return in_maps, chunk_maps


def kernel(data: np.ndarray, rois: np.ndarray, offset: np.ndarray) -> np.ndarray:
    from concourse.bass_utils import run_bass_kernel_spmd

    data = np.ascontiguousarray(data, dtype=np.float32)
    rois = np.asarray(rois, dtype=np.float32)
    offset = np.asarray(offset, dtype=np.float32)
    R = rois.shape[0]

    per_roi = _host_tables(rois, offset)
    per_seg = _segments(per_roi)
    core_rois = _balance(per_seg)
    C = 0
    for c in range(N_CORES):
        _, rows = _pack_core(core_rois[c], per_seg)
        C = max(C, (rows + CHUNK - 1) // CHUNK)
    nc = _build_program(C)

    # channel-last quad-row layout, both images stacked + KSEG zero pad rows
    dataT = np.concatenate([
        np.ascontiguousarray(data[i].transpose(1, 2, 0)).reshape(NQROWS, ROW_ELEMS)
        for i in range(N_IMG)
    ] + [np.zeros((KSEG, ROW_ELEMS), np.float32)], axis=0)
    dataT_b = _to_bf16(dataT)
    in_maps, chunk_maps = _core_inputs(per_seg, core_rois, C, dataT_b)

    res = run_bass_kernel_spmd(nc, in_maps, list(range(N_CORES)), trace=False)

    out_full = np.zeros((R, C_FEAT, P, P), np.float32)
    for c in range(N_CORES):
        o = np.asarray(res.results[c]["out"]).astype(np.float32)  # [98, C*256]
        o = o.reshape(MBLK, C, C_FEAT).transpose(1, 0, 2)         # [C, 98, 256]
        for k in range(C):
            for beta in range(2):
                rid = chunk_maps[c][k][beta]
                if rid >= 0:
                    blk = o[k, beta * NBINS:(beta + 1) * NBINS]   # [49, 256]
                    out_full[rid] += blk.T.reshape(C_FEAT, P, P)
    return out_full
